# revision 7
# baseline (speedup 1.0000x reference)
"""CSPN 3x3 propagation step on 8 Trainium2 NeuronCores.

out[b,0,r,c] = sum_k aff[b,k,r,c] * patch_k(cur)[r,c], with the center tap
(k=4) taken from coarse_seg instead of cur_seg. Zero padding at image edges.

Sharding: pure data parallel over batch (16 images -> 2 per core), one SPMD
Bass program run on all 8 cores with per-core input slices.

Per-core algorithm (per 512x512 image): rows are packed PARTITION-MAJOR,
r = 4p + t  (partition p in 0..127, sub-row t in 0..3), so a +-1 row shift
stays inside the partition (a free-dim offset) for 3 of the 4 sub-rows.
The block-edge rows (r = 4p-1 and r = 4p+4) are covered by two small
[128, 512] edge-plane loads (stride-4 row gather from HBM, 256 KB each).

  - No TensorEngine, no PSUM, no evacuation: the whole kernel is DMA plus
    elementwise multiply/add split across DVE and Pool (GpSimd).
  - THREE DMA channels run concurrently: the ACT and SP HWDGE rings carry
    cur/coarse/edges/out plus 6 affinity planes (8 KB descriptors); the
    center group's planes (k=3,4,5) ride ONE Pool software-DGE dma_start
    (a single 3 MB blob, ~1.2 us of Pool engine time to issue).
  - Taps: product P_k = aff_k * shifted cur (dx = free-dim column offset
    into zero-padded tiles; dy = sub-row offset or edge plane), center tap
    k=4 multiplies coarse_seg. Tree-sum, split-half finals, DMA store.
"""

import sys

import numpy as np

if "/opt/trn_rl_repo" not in sys.path:
    sys.path.insert(0, "/opt/trn_rl_repo")

B_PER_CORE = 2
N_CORES = 8
H = 512
W = 512
NBLK = 4  # sub-rows per partition
WPAD = W + 2  # zero column on each side

_compiled = None
_compiled_reps = {}


def _build_program(reps=1):
    """reps>1 unrolls the whole per-core computation `reps` times inside one
    NEFF — used only to measure kernel time through the dispatch noise."""
    import concourse.bacc as bacc
    import concourse.mybir as mybir
    import concourse.tile as tile

    fp32 = mybir.dt.float32

    nc = bacc.Bacc(
        "TRN2",
        target_bir_lowering=False,
        debug=False,
        enable_asserts=False,
        num_devices=N_CORES,
    )

    aff_d = nc.dram_tensor(
        "affinity", [B_PER_CORE, 9, H, W], fp32, kind="ExternalInput"
    ).ap()
    cur_d = nc.dram_tensor(
        "cur_seg", [B_PER_CORE, 1, H, W], fp32, kind="ExternalInput"
    ).ap()
    coa_d = nc.dram_tensor(
        "coarse_seg", [B_PER_CORE, 1, H, W], fp32, kind="ExternalInput"
    ).ap()
    out_d = nc.dram_tensor(
        "out", [B_PER_CORE, 1, H, W], fp32, kind="ExternalOutput"
    ).ap()

    with tile.TileContext(nc) as tc:
        with (
            tc.tile_pool(name="cur", bufs=2) as cur_pool,
            tc.tile_pool(name="edge", bufs=4) as edge_pool,
            tc.tile_pool(name="coa", bufs=2) as coa_pool,
            tc.tile_pool(name="aff", bufs=7) as aff_pool,
            tc.tile_pool(name="blob", bufs=2) as blob_pool,
            tc.tile_pool(name="prod", bufs=6) as prod_pool,
        ):
            for b in [bb for _ in range(reps) for bb in range(B_PER_CORE)]:
                # DMA channel plan per image (transfers serialize per channel):
                #   ACT ring:  tM, ak7, ak8, ak1, tU, out[0:2]
                #   SP ring:   ak6, ak0, ak2, tD, tC, out[2:4]
                #   Pool swdge: aff[3:6] as one 3 MB blob (center group)

                def _load_aff(k, ring):
                    ak = aff_pool.tile([128, NBLK, W], fp32, tag="aff")
                    ring.dma_start(
                        out=ak[:],
                        in_=aff_d[b, k].rearrange("(p t) c -> p t c", p=128),
                    )
                    return ak

                # --- cur tile [128, 4, 514]: [p, t, 1+c] = cur[4p+t, c] ---
                tM = cur_pool.tile([128, NBLK, WPAD], fp32, tag="cur")
                nc.vector.memset(tM[:, :, 0:1], 0.0)
                nc.vector.memset(tM[:, :, WPAD - 1 : WPAD], 0.0)
                cur_rows = cur_d[b, 0].rearrange("(p t) c -> p t c", p=128)
                nc.scalar.dma_start(out=tM[:, :, 1 : W + 1], in_=cur_rows)

                a6 = _load_aff(6, nc.sync)
                a7 = _load_aff(7, nc.scalar)

                # --- center-group blob [128, 3, 4, 512]: planes k=3,4,5 ---
                ab = blob_pool.tile([128, 3, NBLK, W], fp32, tag="blob")
                nc.gpsimd.dma_start(
                    out=ab[:],
                    in_=aff_d[b, 3:6].rearrange("k (p t) c -> p k t c", p=128),
                )

                # --- edge planes [128, 514]: tD[p] = cur[4p-1], tU[p] = cur[4p+4]
                tD = edge_pool.tile([128, WPAD], fp32, tag="ed")
                nc.gpsimd.memset(tD[:], 0.0)
                dn_rows = cur_d[b, 0][3 : H - 1].rearrange("(p t) c -> p t c", t=4)
                nc.sync.dma_start(out=tD[1:128, 1 : W + 1], in_=dn_rows[:, 0, :])

                a8 = _load_aff(8, nc.scalar)
                a0 = _load_aff(0, nc.sync)

                tU = edge_pool.tile([128, WPAD], fp32, tag="eu")
                nc.gpsimd.memset(tU[:], 0.0)
                up_rows = cur_d[b, 0][4:H].rearrange("(p t) c -> p t c", t=4)
                nc.scalar.dma_start(out=tU[0:127, 1 : W + 1], in_=up_rows[:, 0, :])

                # dx column windows into the padded tiles
                def mwin(tlo, thi, dxi):
                    return tM[:, tlo:thi, dxi : dxi + W]

                # --- group dy=+1 (k=6,7,8): patch row r+1 = [p, t+1] or tU ---
                P6 = prod_pool.tile([128, NBLK, W], fp32, tag="prod")
                nc.vector.tensor_mul(out=P6[:, 0:3, :], in0=a6[:, 0:3, :], in1=mwin(1, 4, 0))
                nc.vector.tensor_mul(out=P6[:, 3, :], in0=a6[:, 3, :], in1=tU[:, 0:W])
                P7 = prod_pool.tile([128, NBLK, W], fp32, tag="prod")
                nc.vector.tensor_mul(out=P7[:, 0:3, :], in0=a7[:, 0:3, :], in1=mwin(1, 4, 1))
                nc.gpsimd.tensor_mul(out=P7[:, 3, :], in0=a7[:, 3, :], in1=tU[:, 1 : 1 + W])
                nc.vector.tensor_add(out=P6[:], in0=P6[:], in1=P7[:])
                a1 = _load_aff(1, nc.scalar)
                P8 = prod_pool.tile([128, NBLK, W], fp32, tag="prod")
                nc.gpsimd.tensor_mul(out=P8[:, 0:3, :], in0=a8[:, 0:3, :], in1=mwin(1, 4, 2))
                nc.vector.tensor_mul(out=P8[:, 3, :], in0=a8[:, 3, :], in1=tU[:, 2 : 2 + W])
                nc.gpsimd.tensor_add(out=P6[:], in0=P6[:], in1=P8[:])

                a2 = _load_aff(2, nc.sync)

                # --- group dy=-1 (k=0,1,2): patch row r-1 = [p, t-1] or tD ---
                P0 = prod_pool.tile([128, NBLK, W], fp32, tag="prod")
                nc.gpsimd.tensor_mul(out=P0[:, 1:4, :], in0=a0[:, 1:4, :], in1=mwin(0, 3, 0))
                nc.vector.tensor_mul(out=P0[:, 0, :], in0=a0[:, 0, :], in1=tD[:, 0:W])
                P1 = prod_pool.tile([128, NBLK, W], fp32, tag="prod")
                nc.gpsimd.tensor_mul(out=P1[:, 1:4, :], in0=a1[:, 1:4, :], in1=mwin(0, 3, 1))
                nc.vector.tensor_mul(out=P1[:, 0, :], in0=a1[:, 0, :], in1=tD[:, 1 : 1 + W])
                nc.vector.tensor_add(out=P0[:], in0=P0[:], in1=P1[:])
                P2 = prod_pool.tile([128, NBLK, W], fp32, tag="prod")
                nc.gpsimd.tensor_mul(out=P2[:, 1:4, :], in0=a2[:, 1:4, :], in1=mwin(0, 3, 2))
                nc.vector.tensor_mul(out=P2[:, 0, :], in0=a2[:, 0, :], in1=tD[:, 2 : 2 + W])
                nc.gpsimd.tensor_add(out=P0[:], in0=P0[:], in1=P2[:])

                # --- coarse tile [128, 4, 512] (center tap, no shift) ---
                tC = coa_pool.tile([128, NBLK, W], fp32, tag="coa")
                nc.sync.dma_start(
                    out=tC[:], in_=coa_d[b, 0].rearrange("(p t) c -> p t c", p=128)
                )

                # --- group dy=0 (k=3,4,5) from the blob; k=4 uses coarse ---
                p3 = prod_pool.tile([128, NBLK, W], fp32, tag="prod")
                nc.vector.tensor_mul(out=p3[:], in0=ab[:, 0], in1=mwin(0, 4, 0))
                p4 = prod_pool.tile([128, NBLK, W], fp32, tag="prod")
                nc.gpsimd.tensor_mul(out=p4[:], in0=ab[:, 1], in1=tC[:])
                nc.vector.tensor_add(out=p3[:], in0=p3[:], in1=p4[:])
                p5 = prod_pool.tile([128, NBLK, W], fp32, tag="prod")
                nc.gpsimd.tensor_mul(out=p5[:], in0=ab[:, 2], in1=mwin(0, 4, 2))
                nc.gpsimd.tensor_add(out=p3[:], in0=p3[:], in1=p5[:])

                # --- final sum + store, halves crossed over both engines ---
                out_rows = out_d[b, 0].rearrange("(p t) c -> p t c", p=128)
                nc.vector.tensor_add(
                    out=P6[:, 0:2, :], in0=P6[:, 0:2, :], in1=P0[:, 0:2, :]
                )
                nc.gpsimd.tensor_add(
                    out=P6[:, 0:2, :], in0=P6[:, 0:2, :], in1=p3[:, 0:2, :]
                )
                nc.scalar.dma_start(out=out_rows[:, 0:2, :], in_=P6[:, 0:2, :])
                nc.gpsimd.tensor_add(
                    out=P6[:, 2:4, :], in0=P6[:, 2:4, :], in1=P0[:, 2:4, :]
                )
                nc.vector.tensor_add(
                    out=P6[:, 2:4, :], in0=P6[:, 2:4, :], in1=p3[:, 2:4, :]
                )
                nc.sync.dma_start(out=out_rows[:, 2:4, :], in_=P6[:, 2:4, :])

    nc.compile()
    return nc


def _get_program(reps=1):
    global _compiled
    if reps != 1:
        if reps not in _compiled_reps:
            _compiled_reps[reps] = _build_program(reps)
        return _compiled_reps[reps]
    if _compiled is None:
        _compiled = _build_program()
    return _compiled


def _in_maps(affinity, cur_seg, coarse_seg):
    maps = []
    for j in range(N_CORES):
        s = slice(j * B_PER_CORE, (j + 1) * B_PER_CORE)
        maps.append(
            {
                "affinity": np.ascontiguousarray(affinity[s]),
                "cur_seg": np.ascontiguousarray(cur_seg[s]),
                "coarse_seg": np.ascontiguousarray(coarse_seg[s]),
            }
        )
    return maps


def kernel(affinity, cur_seg, coarse_seg, i=None, **_unused):
    from concourse.bass_utils import run_bass_kernel_spmd

    nc = _get_program()

    affinity = np.ascontiguousarray(affinity, dtype=np.float32)
    cur_seg = np.ascontiguousarray(cur_seg, dtype=np.float32)
    coarse_seg = np.ascontiguousarray(coarse_seg, dtype=np.float32)

    res = run_bass_kernel_spmd(
        nc, _in_maps(affinity, cur_seg, coarse_seg), core_ids=list(range(N_CORES))
    )
    out = np.concatenate([r["out"] for r in res.results], axis=0)
    return out


# revision 10
# speedup vs baseline: 1.1503x; 1.1503x over previous
"""CSPN 3x3 propagation step on 8 Trainium2 NeuronCores.

out[b,0,r,c] = sum_k aff[b,k,r,c] * patch_k(cur)[r,c], with the center tap
(k=4) taken from coarse_seg instead of cur_seg. Zero padding at image edges.

Sharding: pure data parallel over batch (16 images -> 2 per core), one SPMD
Bass program run on all 8 cores with per-core input slices.

Per-core algorithm (per 512x512 image): rows are packed PARTITION-MAJOR,
r = 4p + t  (partition p in 0..127, sub-row t in 0..3), so a +-1 row shift
stays inside the partition (a free-dim offset) for 3 of the 4 sub-rows.
The block-edge rows (r = 4p-1 and r = 4p+4) are covered by two small
[128, 512] edge-plane loads (stride-4 row gather from HBM, 256 KB each).

  - No TensorEngine, no PSUM, no evacuation: the whole kernel is DMA on the
    two HWDGE rings (8 KB descriptors for affinity/coarse/out) plus
    elementwise multiply/add split across DVE and Pool (GpSimd).
  - Zero-pad regions (columns 0/513, edge-plane boundary rows) live in
    PERSISTENT double-buffered tiles memset once before the image loop, so
    no per-image memsets gate the DMA streams.
  - Taps: product P_k = aff_k * shifted cur (dx = free-dim column offset
    into zero-padded tiles; dy = sub-row offset or edge plane), center tap
    k=4 multiplies coarse_seg. Tree-sum; the last plane (k=5) is loaded as
    two ring-split halves and folded directly into the per-half final sums
    so the post-DMA tail stays shallow.
"""

import sys

import numpy as np

if "/opt/trn_rl_repo" not in sys.path:
    sys.path.insert(0, "/opt/trn_rl_repo")

B_PER_CORE = 2
N_CORES = 8
H = 512
W = 512
NBLK = 4  # sub-rows per partition
WPAD = W + 2  # zero column on each side

_compiled = None
_compiled_reps = {}


def _build_program(reps=1):
    """reps>1 unrolls the whole per-core computation `reps` times inside one
    NEFF — used only to measure kernel time through the dispatch noise."""
    import concourse.bacc as bacc
    import concourse.mybir as mybir
    import concourse.tile as tile

    fp32 = mybir.dt.float32

    nc = bacc.Bacc(
        "TRN2",
        target_bir_lowering=False,
        debug=False,
        enable_asserts=False,
        num_devices=N_CORES,
    )

    aff_d = nc.dram_tensor(
        "affinity", [B_PER_CORE, 9, H, W], fp32, kind="ExternalInput"
    ).ap()
    cur_d = nc.dram_tensor(
        "cur_seg", [B_PER_CORE, 1, H, W], fp32, kind="ExternalInput"
    ).ap()
    coa_d = nc.dram_tensor(
        "coarse_seg", [B_PER_CORE, 1, H, W], fp32, kind="ExternalInput"
    ).ap()
    out_d = nc.dram_tensor(
        "out", [B_PER_CORE, 1, H, W], fp32, kind="ExternalOutput"
    ).ap()

    with tile.TileContext(nc) as tc:
        with (
            tc.tile_pool(name="cur", bufs=2) as cur_pool,
            tc.tile_pool(name="edge", bufs=4) as edge_pool,
            tc.tile_pool(name="coa", bufs=2) as coa_pool,
            tc.tile_pool(name="aff", bufs=8) as aff_pool,
            tc.tile_pool(name="prod", bufs=6) as prod_pool,
            tc.tile_pool(name="p5h", bufs=2) as p5h_pool,
        ):
            # Persistent double-buffered tiles: pad regions zeroed ONCE here;
            # per-image DMAs only overwrite the data regions.
            tM_bufs, tU_bufs, tD_bufs = [], [], []
            for par in range(2):
                tM = cur_pool.tile([128, NBLK, WPAD], fp32, name=f"tM{par}")
                nc.vector.memset(tM[:, :, 0:1], 0.0)
                nc.vector.memset(tM[:, :, WPAD - 1 : WPAD], 0.0)
                tM_bufs.append(tM)
                tU = edge_pool.tile([128, WPAD], fp32, name=f"tU{par}")
                nc.gpsimd.memset(tU[:], 0.0)
                tU_bufs.append(tU)
                tD = edge_pool.tile([128, WPAD], fp32, name=f"tD{par}")
                nc.gpsimd.memset(tD[:], 0.0)
                tD_bufs.append(tD)

            for i, b in enumerate(
                [bb for _ in range(reps) for bb in range(B_PER_CORE)]
            ):
                # DMA issue order per ring (transfers serialize per ring):
                #   ACT: tM, ak7, ak8, tU, ak1, ak3, ak5[2:4], out[0:2]
                #   SP:  ak6, ak0, tD, ak2, tC, ak4, ak5[0:2], out[2:4]
                par = i % 2

                def _load_aff(k, ring):
                    ak = aff_pool.tile([128, NBLK, W], fp32, tag="aff")
                    ring.dma_start(
                        out=ak[:],
                        in_=aff_d[b, k].rearrange("(p t) c -> p t c", p=128),
                    )
                    return ak

                # --- cur tile [128, 4, 514]: [p, t, 1+c] = cur[4p+t, c] ---
                tM = tM_bufs[par]
                cur_rows = cur_d[b, 0].rearrange("(p t) c -> p t c", p=128)
                nc.scalar.dma_start(out=tM[:, :, 1 : W + 1], in_=cur_rows)

                a6 = _load_aff(6, nc.sync)
                a7 = _load_aff(7, nc.scalar)
                a0 = _load_aff(0, nc.sync)
                a8 = _load_aff(8, nc.scalar)

                # --- edge planes [128, 514]: tD[p] = cur[4p-1], tU[p] = cur[4p+4]
                tD = tD_bufs[par]
                dn_rows = cur_d[b, 0][3 : H - 1].rearrange("(p t) c -> p t c", t=4)
                nc.sync.dma_start(out=tD[1:128, 1 : W + 1], in_=dn_rows[:, 0, :])

                tU = tU_bufs[par]
                up_rows = cur_d[b, 0][4:H].rearrange("(p t) c -> p t c", t=4)
                nc.scalar.dma_start(out=tU[0:127, 1 : W + 1], in_=up_rows[:, 0, :])

                # dx column windows into the padded tiles
                def mwin(tlo, thi, dxi):
                    return tM[:, tlo:thi, dxi : dxi + W]

                # --- group dy=+1 (k=6,7,8): patch row r+1 = [p, t+1] or tU ---
                P6 = prod_pool.tile([128, NBLK, W], fp32, tag="prod")
                nc.vector.tensor_mul(out=P6[:, 0:3, :], in0=a6[:, 0:3, :], in1=mwin(1, 4, 0))
                nc.vector.tensor_mul(out=P6[:, 3, :], in0=a6[:, 3, :], in1=tU[:, 0:W])
                P7 = prod_pool.tile([128, NBLK, W], fp32, tag="prod")
                nc.vector.tensor_mul(out=P7[:, 0:3, :], in0=a7[:, 0:3, :], in1=mwin(1, 4, 1))
                nc.gpsimd.tensor_mul(out=P7[:, 3, :], in0=a7[:, 3, :], in1=tU[:, 1 : 1 + W])
                nc.vector.tensor_add(out=P6[:], in0=P6[:], in1=P7[:])
                a1 = _load_aff(1, nc.scalar)
                P8 = prod_pool.tile([128, NBLK, W], fp32, tag="prod")
                nc.gpsimd.tensor_mul(out=P8[:, 0:3, :], in0=a8[:, 0:3, :], in1=mwin(1, 4, 2))
                nc.gpsimd.tensor_mul(out=P8[:, 3, :], in0=a8[:, 3, :], in1=tU[:, 2 : 2 + W])
                nc.gpsimd.tensor_add(out=P6[:], in0=P6[:], in1=P8[:])

                a2 = _load_aff(2, nc.sync)

                # --- group dy=-1 (k=0,1,2): patch row r-1 = [p, t-1] or tD ---
                P0 = prod_pool.tile([128, NBLK, W], fp32, tag="prod")
                nc.gpsimd.tensor_mul(out=P0[:, 1:4, :], in0=a0[:, 1:4, :], in1=mwin(0, 3, 0))
                nc.vector.tensor_mul(out=P0[:, 0, :], in0=a0[:, 0, :], in1=tD[:, 0:W])
                P1 = prod_pool.tile([128, NBLK, W], fp32, tag="prod")
                nc.gpsimd.tensor_mul(out=P1[:, 1:4, :], in0=a1[:, 1:4, :], in1=mwin(0, 3, 1))
                nc.vector.tensor_mul(out=P1[:, 0, :], in0=a1[:, 0, :], in1=tD[:, 1 : 1 + W])
                nc.vector.tensor_add(out=P0[:], in0=P0[:], in1=P1[:])
                P2 = prod_pool.tile([128, NBLK, W], fp32, tag="prod")
                nc.gpsimd.tensor_mul(out=P2[:, 1:4, :], in0=a2[:, 1:4, :], in1=mwin(0, 3, 2))
                nc.vector.tensor_mul(out=P2[:, 0, :], in0=a2[:, 0, :], in1=tD[:, 2 : 2 + W])
                nc.gpsimd.tensor_add(out=P0[:], in0=P0[:], in1=P2[:])

                # --- coarse tile + center group k=3,4 (k=4 uses coarse) ---
                a3 = _load_aff(3, nc.scalar)
                tC = coa_pool.tile([128, NBLK, W], fp32, tag="coa")
                nc.sync.dma_start(
                    out=tC[:], in_=coa_d[b, 0].rearrange("(p t) c -> p t c", p=128)
                )
                a4 = _load_aff(4, nc.sync)

                p3 = prod_pool.tile([128, NBLK, W], fp32, tag="prod")
                nc.vector.tensor_mul(out=p3[:], in0=a3[:], in1=mwin(0, 4, 0))
                p4 = prod_pool.tile([128, NBLK, W], fp32, tag="prod")
                nc.gpsimd.tensor_mul(out=p4[:], in0=a4[:], in1=tC[:])
                nc.gpsimd.tensor_add(out=p3[:], in0=p3[:], in1=p4[:])

                # --- k=5 as ring-split halves, folded into per-half finals ---
                a5 = aff_pool.tile([128, NBLK, W], fp32, tag="aff")
                a5_rows = aff_d[b, 5].rearrange("(p t) c -> p t c", p=128)
                nc.sync.dma_start(out=a5[:, 0:2, :], in_=a5_rows[:, 0:2, :])
                nc.scalar.dma_start(out=a5[:, 2:4, :], in_=a5_rows[:, 2:4, :])

                out_rows = out_d[b, 0].rearrange("(p t) c -> p t c", p=128)
                # lower half: Z = (P6+P0)+p3, then += a5*cur, store on ACT
                nc.vector.tensor_add(
                    out=P6[:, 0:2, :], in0=P6[:, 0:2, :], in1=P0[:, 0:2, :]
                )
                nc.gpsimd.tensor_add(
                    out=P6[:, 0:2, :], in0=P6[:, 0:2, :], in1=p3[:, 0:2, :]
                )
                p5lo = p5h_pool.tile([128, 2, W], fp32, tag="p5h")
                nc.gpsimd.tensor_mul(
                    out=p5lo[:], in0=a5[:, 0:2, :], in1=tM[:, 0:2, 2 : 2 + W]
                )
                nc.vector.tensor_add(out=P6[:, 0:2, :], in0=P6[:, 0:2, :], in1=p5lo[:])
                nc.scalar.dma_start(out=out_rows[:, 0:2, :], in_=P6[:, 0:2, :])
                # upper half: mirrored engines, store on SP
                nc.gpsimd.tensor_add(
                    out=P6[:, 2:4, :], in0=P6[:, 2:4, :], in1=P0[:, 2:4, :]
                )
                nc.vector.tensor_add(
                    out=P6[:, 2:4, :], in0=P6[:, 2:4, :], in1=p3[:, 2:4, :]
                )
                p5hi = p5h_pool.tile([128, 2, W], fp32, tag="p5h")
                nc.vector.tensor_mul(
                    out=p5hi[:], in0=a5[:, 2:4, :], in1=tM[:, 2:4, 2 : 2 + W]
                )
                nc.gpsimd.tensor_add(out=P6[:, 2:4, :], in0=P6[:, 2:4, :], in1=p5hi[:])
                nc.sync.dma_start(out=out_rows[:, 2:4, :], in_=P6[:, 2:4, :])

    nc.compile()
    return nc


def _get_program(reps=1):
    global _compiled
    if reps != 1:
        if reps not in _compiled_reps:
            _compiled_reps[reps] = _build_program(reps)
        return _compiled_reps[reps]
    if _compiled is None:
        _compiled = _build_program()
    return _compiled


def _in_maps(affinity, cur_seg, coarse_seg):
    maps = []
    for j in range(N_CORES):
        s = slice(j * B_PER_CORE, (j + 1) * B_PER_CORE)
        maps.append(
            {
                "affinity": np.ascontiguousarray(affinity[s]),
                "cur_seg": np.ascontiguousarray(cur_seg[s]),
                "coarse_seg": np.ascontiguousarray(coarse_seg[s]),
            }
        )
    return maps


def kernel(affinity, cur_seg, coarse_seg, i=None, **_unused):
    from concourse.bass_utils import run_bass_kernel_spmd

    nc = _get_program()

    affinity = np.ascontiguousarray(affinity, dtype=np.float32)
    cur_seg = np.ascontiguousarray(cur_seg, dtype=np.float32)
    coarse_seg = np.ascontiguousarray(coarse_seg, dtype=np.float32)

    res = run_bass_kernel_spmd(
        nc, _in_maps(affinity, cur_seg, coarse_seg), core_ids=list(range(N_CORES))
    )
    out = np.concatenate([r["out"] for r in res.results], axis=0)
    return out
